# revision 13
# baseline (speedup 1.0000x reference)
"""Causal multi-head attention block (qkv proj + attention + out proj) on 8 TRN2 cores.

Sharding: core c = (batch b = c//2, head-group hg = c%2 of 8 heads).
Each core computes, for its (b, hg): qk projection (transposed), v projection,
per-head causal softmax attention (transposed layout, denominator folded into
the PV matmul as an extra ones-row of v), and the partial output projection
over its 512 attention dims. Host sums the two head-group partials per batch
and adds the bias.
"""

import numpy as np
import ml_dtypes

import concourse.bass as bass
import concourse.bacc as bacc
import concourse.mybir as mybir
import concourse.tile as tile
from concourse.bass_utils import run_bass_kernel_spmd

BF = ml_dtypes.bfloat16

B, T, D, H = 4, 2048, 1024, 16
HD = 64          # head dim
HL = 8           # heads per core
DL = 512         # attention dims per core
NKT = T // 128   # 16 k-tiles
NQC = T // 512   # 4 q-chunks
SCALE = HD ** -0.5

F32 = mybir.dt.float32
F32R = mybir.dt.float32r
BF16 = mybir.dt.bfloat16
EXP = mybir.ActivationFunctionType.Exp

_cache = {}


def _build_nc():
    nc = bacc.Bacc("TRN2", target_bir_lowering=False, debug=False, num_devices=8)

    xT_d = nc.dram_tensor("xT", [D, T], BF16, kind="ExternalInput").ap()
    wqk_d = nc.dram_tensor("wqk", [D, 2 * DL], BF16, kind="ExternalInput").ap()
    wv_d = nc.dram_tensor("wv", [D, DL], BF16, kind="ExternalInput").ap()
    wo_d = nc.dram_tensor("wo", [DL, D], BF16, kind="ExternalInput").ap()
    cmask_d = nc.dram_tensor("cmask", [128, 4 * 512], BF16, kind="ExternalInput").ap()
    sel_d = nc.dram_tensor("sel", [128, 4 * 64], F32, kind="ExternalInput").ap()
    outT_d = nc.dram_tensor("outT", [D, T], F32, kind="ExternalOutput").ap()

    with tile.TileContext(nc) as tc:
        with (
            tc.tile_pool(name="persist", bufs=1) as pp,
            tc.tile_pool(name="work", bufs=1) as wp,
            tc.tile_pool(name="ps_m", bufs=2, space="PSUM") as ps_m,
            tc.tile_pool(name="ps_s", bufs=1, space="PSUM") as ps_s,
            tc.tile_pool(name="ps_o", bufs=4, space="PSUM") as ps_o,
        ):
            # ---- resident inputs -------------------------------------------------
            xT_sb = [pp.tile([128, T], BF16, tag=f"xT{i}", name=f"xT{i}") for i in range(8)]
            wqk_sb = [pp.tile([128, 2 * DL], BF16, tag=f"wqk{i}", name=f"wqk{i}") for i in range(8)]
            wv_sb = [pp.tile([128, DL], BF16, tag=f"wv{i}", name=f"wv{i}") for i in range(8)]
            wo_sb = [pp.tile([128, D], BF16, tag=f"wo{i}", name=f"wo{i}") for i in range(4)]
            cmask_sb = pp.tile([128, 4 * 512], BF16, tag="cmask", name="cmask")
            sel_sb = pp.tile([128, 4 * 64], F32R, tag="sel", name="sel")

            for i in range(8):
                nc.sync.dma_start(wqk_sb[i][:], wqk_d[i * 128:(i + 1) * 128, :])
            for i in range(8):
                nc.sync.dma_start(xT_sb[i][:], xT_d[i * 128:(i + 1) * 128, :])
            for i in range(8):
                nc.sync.dma_start(wv_sb[i][:], wv_d[i * 128:(i + 1) * 128, :])
            for i in range(4):
                nc.sync.dma_start(wo_sb[i][:], wo_d[i * 128:(i + 1) * 128, :])
            nc.sync.dma_start(cmask_sb[:], cmask_d)
            nc.sync.dma_start(sel_sb[:], sel_d.bitcast(F32R))

            # ---- persistent intermediates ---------------------------------------
            qkT_sb = [pp.tile([128, T], BF16, tag=f"qkT{i}", name=f"qkT{i}") for i in range(8)]
            vaug_sb = [pp.tile([128, HL, HD + 1], BF16, tag=f"vaug{i}", name=f"vaug{i}") for i in range(NKT)]
            attnT_sb = [pp.tile([128, T], BF16, tag=f"attnT{i}", name=f"attnT{i}") for i in range(4)]

            for i in range(NKT):
                nc.vector.memset(vaug_sb[i][:, :, HD:HD + 1], 1.0)

            # ---- qk projection: qkT[m, t] = sum_d wqk[d, m] * xT[d, t] ----------
            for mt in range(8):
                for qc in range(NQC):
                    ps = ps_m.tile([128, 512], F32, tag="ps_m", name="ps_m")
                    for kt in range(8):
                        nc.tensor.matmul(
                            ps[:],
                            wqk_sb[kt][:, mt * 128:(mt + 1) * 128],
                            xT_sb[kt][:, qc * 512:(qc + 1) * 512],
                            start=(kt == 0), stop=(kt == 7),
                        )
                    nc.vector.tensor_copy(
                        qkT_sb[mt][:, qc * 512:(qc + 1) * 512], ps[:]
                    )

            # ---- v projection: v[t, n] = sum_d xT[d, t] * wv[d, n] --------------
            for tt in range(NKT):
                ps = ps_m.tile([128, 512], F32, tag="ps_m", name="ps_m")
                for kt in range(8):
                    nc.tensor.matmul(
                        ps[:],
                        xT_sb[kt][:, tt * 128:(tt + 1) * 128],
                        wv_sb[kt][:],
                        start=(kt == 0), stop=(kt == 7),
                    )
                nc.vector.tensor_copy(
                    vaug_sb[tt][:, :, 0:HD],
                    ps[:].rearrange("p (h e) -> p h e", h=HL),
                )

            # ---- attention, one head at a time ----------------------------------
            for h in range(HL):
                hp, ho = h // 2, (h % 2) * 64
                qT = qkT_sb[hp]
                kT = qkT_sb[4 + hp]
                # denominator rows land at 32-aligned partitions (engine
                # partition windows must start at a multiple of 32)
                den = wp.tile([128, 512], F32, tag="den", bufs=2, name="den")
                nc.vector.memset(den[:], 1.0)
                po = {}
                for qc2 in range(2):
                    qcs = [2 * qc2, 2 * qc2 + 1]
                    for qc in qcs:
                        po[qc] = ps_o.tile([128, 512], F32, tag="ps_o", name="ps_o")
                    for kt in range(4 * qcs[1] + 4):
                        valid = [qc for qc in qcs if kt <= 4 * qc + 3]
                        ps = ps_s.tile([128, 1024], F32, tag="ps_s", name="ps_s")
                        for qc in valid:
                            off = (qc - qcs[0]) * 512
                            nc.tensor.matmul(
                                ps[:, off:off + 512],
                                kT[ho:ho + 64, kt * 128:(kt + 1) * 128],
                                qT[ho:ho + 64, qc * 512:(qc + 1) * 512],
                                start=True, stop=True,
                            )
                        lo = (valid[0] - qcs[0]) * 512
                        et = wp.tile([128, 1024], BF16, tag="expT", bufs=3, name="expT")
                        nc.scalar.activation(et[:, lo:1024], ps[:, lo:1024], EXP)
                        for qc in valid:
                            j = kt - 4 * qc
                            if 0 <= j <= 3:
                                off = (qc - qcs[0]) * 512
                                nc.vector.tensor_mul(
                                    et[:, off:off + 512],
                                    et[:, off:off + 512],
                                    cmask_sb[:, j * 512:(j + 1) * 512],
                                )
                        for qc in valid:
                            off = (qc - qcs[0]) * 512
                            nc.tensor.matmul(
                                po[qc][0:HD + 1, :],
                                vaug_sb[kt][:, h, :],
                                et[:, off:off + 512],
                                start=(kt == 0), stop=(kt == 4 * qc + 3),
                            )
                            if kt == 4 * qc + 3:
                                nc.vector.tensor_copy(
                                    den[32 * qc:32 * qc + 1, :], po[qc][HD:HD + 1, :]
                                )
                # normalize the whole head: batched exact reciprocal, then
                # selector-matmul broadcast of row qc to 64 partitions
                rec = wp.tile([128, 512], F32R, tag="rec", bufs=2, name="rec")
                with nc.allow_low_precision(reason="f32r reciprocal for denominator broadcast"):
                    nc.vector.reciprocal(rec[:], den[:])
                for qc in range(NQC):
                    pb = ps_m.tile([128, 512], F32, tag="ps_m", name="ps_m")
                    nc.tensor.matmul(
                        pb[0:64, :],
                        sel_sb[:, qc * 64:(qc + 1) * 64],
                        rec[:],
                        start=True, stop=True,
                    )
                    bc = wp.tile([64, 512], F32, tag="bc", bufs=3, name="bc")
                    nc.vector.tensor_copy(bc[:], pb[0:64, :])
                    nc.vector.tensor_mul(
                        attnT_sb[hp][ho:ho + 64, qc * 512:(qc + 1) * 512],
                        po[qc][0:64, :],
                        bc[:],
                    )

            # ---- output projection: outT[m, t] = sum_n wo[n, m] * attnT[n, t] ---
            for mt in range(8):
                for qc in range(NQC):
                    ps = ps_m.tile([128, 512], F32, tag="ps_m", name="ps_m")
                    for dt_ in range(4):
                        nc.tensor.matmul(
                            ps[:],
                            wo_sb[dt_][:, mt * 128:(mt + 1) * 128],
                            attnT_sb[dt_][:, qc * 512:(qc + 1) * 512],
                            start=(dt_ == 0), stop=(dt_ == 3),
                        )
                    st = wp.tile([128, 512], F32, tag="outst", bufs=3, name="outst")
                    nc.vector.tensor_copy(st[:], ps[:])
                    nc.sync.dma_start(
                        outT_d[mt * 128:(mt + 1) * 128, qc * 512:(qc + 1) * 512],
                        st[:],
                    )

    nc.compile()
    return nc


def _host_shards(x, qkv_w, out_w):
    cmask = np.zeros((128, 4 * 512), np.float32)
    kp = np.arange(128)[:, None]
    qf = np.arange(512)[None, :]
    for j in range(4):
        cmask[:, j * 512:(j + 1) * 512] = (kp <= qf - 128 * j)
    sel = np.zeros((128, 4 * 64), np.float32)
    for r in range(4):
        sel[32 * r, r * 64:(r + 1) * 64] = 1.0

    in_maps = []
    for c in range(8):
        b, hg = c // 2, c % 2
        xT = np.ascontiguousarray(x[b].T).astype(BF)
        qs = (qkv_w[hg * DL:(hg + 1) * DL] * np.float32(SCALE))
        ks = qkv_w[D + hg * DL:D + (hg + 1) * DL]
        wqk = np.ascontiguousarray(np.concatenate([qs, ks], 0).T).astype(BF)
        wv = np.ascontiguousarray(qkv_w[2 * D + hg * DL:2 * D + (hg + 1) * DL].T).astype(BF)
        wo = np.ascontiguousarray(out_w[:, hg * DL:(hg + 1) * DL].T).astype(BF)
        in_maps.append({
            "xT": xT,
            "wqk": wqk,
            "wv": wv,
            "wo": wo,
            "cmask": cmask.astype(BF),
            "sel": sel,
        })
    return in_maps


def kernel(x, qkv_w, out_w, out_b, _trace=False, _trace_kwargs=None):
    if "nc" not in _cache:
        _cache["nc"] = _build_nc()
    nc = _cache["nc"]
    in_maps = _host_shards(
        np.asarray(x, np.float32), np.asarray(qkv_w, np.float32),
        np.asarray(out_w, np.float32),
    )
    res = run_bass_kernel_spmd(
        nc, in_maps, core_ids=list(range(8)), trace=_trace,
        **(_trace_kwargs or {}),
    )
    _cache["last_result"] = res
    out = np.empty((B, T, D), np.float32)
    ob = np.asarray(out_b, np.float32)[None, :]
    for b in range(B):
        acc = res.results[2 * b]["outT"] + res.results[2 * b + 1]["outT"]
        out[b] = acc.T + ob
    return out


# revision 18
# speedup vs baseline: 1.4353x; 1.4353x over previous
"""Causal multi-head attention block (qkv proj + attention + out proj) on 8 TRN2 cores.

Sharding: core c = (batch b = c//2, head-group hg = c%2 of 8 heads).
Each core computes, for its (b, hg): qk projection (transposed), v projection,
per-head causal softmax attention (transposed layout, denominator folded into
the PV matmul as an extra ones-row of v), and the partial output projection
over its 512 attention dims. Host sums the two head-group partials per batch
and adds the bias.

Attention processes heads in pairs (even head on partitions 0-63, odd head on
64-127) so the K=64 score matmuls alternate PE row groups and run
concurrently. qk-projection chains for the next pair are interleaved into the
attention steps to keep the PE dense (and HAM-warm) while ACT runs the exps.
"""

import numpy as np
import ml_dtypes

import concourse.bass as bass
import concourse.bacc as bacc
import concourse.mybir as mybir
import concourse.tile as tile
from concourse.bass_utils import run_bass_kernel_spmd

BF = ml_dtypes.bfloat16

B, T, D, H = 4, 2048, 1024, 16
HD = 64          # head dim
HL = 8           # heads per core
DL = 512         # attention dims per core
NKT = T // 128   # 16 k-tiles
NQC = T // 512   # 4 q-chunks
SCALE = HD ** -0.5
MASKVAL = -30000.0

F32 = mybir.dt.float32
F32R = mybir.dt.float32r
BF16 = mybir.dt.bfloat16
EXP = mybir.ActivationFunctionType.Exp

_cache = {}


def _build_nc():
    nc = bacc.Bacc("TRN2", target_bir_lowering=False, debug=False, num_devices=8)

    xT_d = nc.dram_tensor("xT", [D, T], BF16, kind="ExternalInput").ap()
    wqk_d = nc.dram_tensor("wqk", [D, 2 * DL], BF16, kind="ExternalInput").ap()
    wv_d = nc.dram_tensor("wv", [D, DL], BF16, kind="ExternalInput").ap()
    wo_d = nc.dram_tensor("wo", [DL, D], BF16, kind="ExternalInput").ap()
    cmask_d = nc.dram_tensor("cmask", [128, 4 * 512], F32, kind="ExternalInput").ap()
    sel_d = nc.dram_tensor("sel", [128, 4 * 64], F32, kind="ExternalInput").ap()
    outT_d = nc.dram_tensor("outT", [D, T], F32, kind="ExternalOutput").ap()

    with tile.TileContext(nc) as tc:
        with (
            tc.tile_pool(name="persist", bufs=1) as pp,
            tc.tile_pool(name="work", bufs=1) as wp,
            tc.tile_pool(name="ps_m", bufs=2, space="PSUM") as ps_m,
            tc.tile_pool(name="ps_s", bufs=4, space="PSUM") as ps_s,
            tc.tile_pool(name="ps_o", bufs=2, space="PSUM") as ps_o,
        ):
            # ---- resident inputs -------------------------------------------------
            xT_sb = [pp.tile([128, T], BF16, tag=f"xT{i}", name=f"xT{i}") for i in range(8)]
            wqk_sb = [pp.tile([128, 2 * DL], BF16, tag=f"wqk{i}", name=f"wqk{i}") for i in range(8)]
            wv_sb = [pp.tile([128, DL], BF16, tag=f"wv{i}", name=f"wv{i}") for i in range(8)]
            wo_sb = [pp.tile([128, D], BF16, tag=f"wo{i}", name=f"wo{i}") for i in range(4)]
            cmask_sb = pp.tile([128, 4 * 512], F32, tag="cmask", name="cmask")
            sel_sb = pp.tile([128, 4 * 64], F32R, tag="sel", name="sel")

            for i in range(8):
                nc.sync.dma_start(wqk_sb[i][:], wqk_d[i * 128:(i + 1) * 128, :])
            for i in range(8):
                nc.sync.dma_start(xT_sb[i][:], xT_d[i * 128:(i + 1) * 128, :])
            for i in range(8):
                nc.sync.dma_start(wv_sb[i][:], wv_d[i * 128:(i + 1) * 128, :])
            for i in range(4):
                nc.sync.dma_start(wo_sb[i][:], wo_d[i * 128:(i + 1) * 128, :])
            nc.sync.dma_start(cmask_sb[:], cmask_d)
            nc.sync.dma_start(sel_sb[:], sel_d.bitcast(F32R))

            # ---- persistent intermediates ---------------------------------------
            qkT_sb = [pp.tile([128, T], BF16, tag=f"qkT{i}", name=f"qkT{i}") for i in range(8)]
            vaug_sb = [pp.tile([128, HL, HD + 1], BF16, tag=f"vaug{i}", name=f"vaug{i}") for i in range(NKT)]
            attnT_sb = [pp.tile([128, T], BF16, tag=f"attnT{i}", name=f"attnT{i}") for i in range(4)]

            for i in range(NKT):
                nc.vector.memset(vaug_sb[i][:, :, HD:HD + 1], 1.0)

            # one qk-projection output tile: qkT[mt, qc] = sum_d wqk.T x
            def qk_chain(mt, qc):
                ps = ps_m.tile([128, 512], F32, tag="ps_m", name="ps_m")
                for kt in range(8):
                    nc.tensor.matmul(
                        ps[:],
                        wqk_sb[kt][:, mt * 128:(mt + 1) * 128],
                        xT_sb[kt][:, qc * 512:(qc + 1) * 512],
                        start=(kt == 0), stop=(kt == 7),
                    )
                nc.vector.tensor_copy(qkT_sb[mt][:, qc * 512:(qc + 1) * 512], ps[:])

            # pair p needs qkT tiles p (q rows) and 4+p (k rows)
            def qk_chains_for_pair(p):
                for mt in (p, 4 + p):
                    for qc in range(NQC):
                        yield (mt, qc)

            # ---- qk projection for pair 0, then v projection (dense warmup) -----
            for mt, qc in qk_chains_for_pair(0):
                qk_chain(mt, qc)

            for tt in range(NKT):
                ps = ps_m.tile([128, 512], F32, tag="ps_m", name="ps_m")
                for kt in range(8):
                    nc.tensor.matmul(
                        ps[:],
                        xT_sb[kt][:, tt * 128:(tt + 1) * 128],
                        wv_sb[kt][:],
                        start=(kt == 0), stop=(kt == 7),
                    )
                nc.vector.tensor_copy(
                    vaug_sb[tt][:, :, 0:HD],
                    ps[:].rearrange("p (h e) -> p h e", h=HL),
                )

            # ---- attention, head pairs, background qk chains interleaved --------
            # background emitter: one full qk-projection chain (8 matmuls +
            # copy, atomic so its PSUM slot is never recycled mid-chain) for
            # the next pair, spread across this pair's attention steps
            def bg_gen(p):
                if p >= 3:
                    return
                chains = list(qk_chains_for_pair(p + 1))
                nsteps = sum(4 * qc + 4 for qc in range(NQC))  # 40
                done = 0
                for step in range(nsteps):
                    while done < len(chains) and (step + 1) * len(chains) >= (done + 1) * nsteps:
                        qk_chain(*chains[done])
                        done += 1
                    yield
                while done < len(chains):
                    qk_chain(*chains[done])
                    done += 1
                while True:
                    yield

            for p in range(4):
                qT = qkT_sb[p]
                kT = qkT_sb[4 + p]
                bg = bg_gen(p)

                dens = {}
                numers = {}
                for ho in (0, 64):
                    dens[ho] = wp.tile([128, 512], F32, tag=f"den{ho}", bufs=2, name=f"den{ho}")
                    nc.vector.memset(dens[ho][:], 1.0)

                for qc in range(NQC):
                    po = {}
                    for ho in (0, 64):
                        po[ho] = ps_o.tile([128, 512], F32, tag="ps_o", name="ps_o")
                    last_kt = 4 * qc + 3
                    for kt in range(last_kt + 1):
                        ets = {}
                        for ho in (0, 64):
                            ps = ps_s.tile([128, 512], F32, tag="ps_s", name="ps_s")
                            ets[ho] = (ps, wp.tile([128, 512], BF16, tag="expT", bufs=4, name="expT"))
                            nc.tensor.matmul(
                                ps[:],
                                kT[ho:ho + 64, kt * 128:(kt + 1) * 128],
                                qT[ho:ho + 64, qc * 512:(qc + 1) * 512],
                                start=True, stop=True,
                            )
                        j = kt - 4 * qc
                        for ho in (0, 64):
                            ps, et = ets[ho]
                            if 0 <= j <= 3:
                                nc.vector.tensor_add(
                                    ps[:], ps[:], cmask_sb[:, j * 512:(j + 1) * 512]
                                )
                            nc.scalar.activation(et[:], ps[:], EXP)
                        # background projection work hides the exp latency
                        next(bg, None)
                        for ho in (0, 64):
                            _, et = ets[ho]
                            h = 2 * p + (1 if ho else 0)
                            nc.tensor.matmul(
                                po[ho][0:HD + 1, :],
                                vaug_sb[kt][:, h, :],
                                et[:],
                                start=(kt == 0), stop=(kt == last_kt),
                            )
                            if kt == last_kt:
                                nm = wp.tile([64, 512], F32, tag=f"nm{ho}_{qc}",
                                             bufs=2, name=f"nm{ho}_{qc}")
                                numers[(ho, qc)] = nm
                                nc.vector.tensor_copy(nm[:], po[ho][0:64, :])
                                nc.vector.tensor_copy(
                                    dens[ho][32 * qc:32 * qc + 1, :],
                                    po[ho][HD:HD + 1, :],
                                )
                # normalize both heads of the pair
                for ho in (0, 64):
                    rec = wp.tile([128, 512], F32R, tag="rec", bufs=2, name="rec")
                    with nc.allow_low_precision(reason="f32r reciprocal for denom broadcast"):
                        nc.vector.reciprocal(rec[:], dens[ho][:])
                    for qc in range(NQC):
                        pb = ps_m.tile([128, 512], F32, tag="ps_m", name="ps_m")
                        nc.tensor.matmul(
                            pb[0:64, :],
                            sel_sb[:, qc * 64:(qc + 1) * 64],
                            rec[:],
                            start=True, stop=True,
                        )
                        nc.vector.tensor_mul(
                            attnT_sb[p][ho:ho + 64, qc * 512:(qc + 1) * 512],
                            numers[(ho, qc)][:],
                            pb[0:64, :],
                        )


            # ---- output projection: outT[m, t] = sum_n wo[n, m] * attnT[n, t] ---
            for mt in range(8):
                for qc in range(NQC):
                    ps = ps_m.tile([128, 512], F32, tag="ps_m", name="ps_m")
                    for dt_ in range(4):
                        nc.tensor.matmul(
                            ps[:],
                            wo_sb[dt_][:, mt * 128:(mt + 1) * 128],
                            attnT_sb[dt_][:, qc * 512:(qc + 1) * 512],
                            start=(dt_ == 0), stop=(dt_ == 3),
                        )
                    st = wp.tile([128, 512], F32, tag="outst", bufs=3, name="outst")
                    nc.vector.tensor_copy(st[:], ps[:])
                    nc.sync.dma_start(
                        outT_d[mt * 128:(mt + 1) * 128, qc * 512:(qc + 1) * 512],
                        st[:],
                    )

    nc.compile()
    return nc


def _host_shards(x, qkv_w, out_w):
    cmask = np.zeros((128, 4 * 512), np.float32)
    kp = np.arange(128)[:, None]
    qf = np.arange(512)[None, :]
    for j in range(4):
        cmask[:, j * 512:(j + 1) * 512] = np.where(kp <= qf - 128 * j, 0.0, MASKVAL)
    sel = np.zeros((128, 4 * 64), np.float32)
    for r in range(4):
        sel[32 * r, r * 64:(r + 1) * 64] = 1.0

    in_maps = []
    for c in range(8):
        b, hg = c // 2, c % 2
        xT = np.ascontiguousarray(x[b].T).astype(BF)
        qs = (qkv_w[hg * DL:(hg + 1) * DL] * np.float32(SCALE))
        ks = qkv_w[D + hg * DL:D + (hg + 1) * DL]
        wqk = np.ascontiguousarray(np.concatenate([qs, ks], 0).T).astype(BF)
        wv = np.ascontiguousarray(qkv_w[2 * D + hg * DL:2 * D + (hg + 1) * DL].T).astype(BF)
        wo = np.ascontiguousarray(out_w[:, hg * DL:(hg + 1) * DL].T).astype(BF)
        in_maps.append({
            "xT": xT,
            "wqk": wqk,
            "wv": wv,
            "wo": wo,
            "cmask": cmask,
            "sel": sel,
        })
    return in_maps


def kernel(x, qkv_w, out_w, out_b, _trace=False, _trace_kwargs=None):
    if "nc" not in _cache:
        _cache["nc"] = _build_nc()
    nc = _cache["nc"]
    in_maps = _host_shards(
        np.asarray(x, np.float32), np.asarray(qkv_w, np.float32),
        np.asarray(out_w, np.float32),
    )
    res = run_bass_kernel_spmd(
        nc, in_maps, core_ids=list(range(8)), trace=_trace,
        **(_trace_kwargs or {}),
    )
    _cache["last_result"] = res
    out = np.empty((B, T, D), np.float32)
    ob = np.asarray(out_b, np.float32)[None, :]
    for b in range(B):
        acc = res.results[2 * b]["outT"] + res.results[2 * b + 1]["outT"]
        out[b] = acc.T + ob
    return out


# revision 19
# speedup vs baseline: 1.4949x; 1.0415x over previous
"""Causal multi-head attention block (qkv proj + attention + out proj) on 8 TRN2 cores.

Sharding: core c = (batch b = c//2, head-group hg = c%2 of 8 heads).
Each core computes, for its (b, hg): qk projection (transposed), v projection,
per-head causal softmax attention (transposed layout, denominator folded into
the PV matmul as an extra ones-row of v), and per-pair partial output
projections over 128 attention dims each. Host sums the 4 pair-partials of
both head-group cores per batch and adds the bias.

Attention processes heads in pairs (even head on partitions 0-63, odd head on
64-127). Score matmuls are K=64 and run in 2-kt bursts of 4 so consecutive
matmuls alternate PE row groups and overlap in the array. Projection chains
(qk for the next pair, out-projection partials for the previous pair, the
tail of the v projection) are interleaved into the attention steps to keep
the PE dense and HAM-warm while ACT runs the exps.
"""

import numpy as np
import ml_dtypes

import concourse.bass as bass
import concourse.bacc as bacc
import concourse.mybir as mybir
import concourse.tile as tile
from concourse.bass_utils import run_bass_kernel_spmd

BF = ml_dtypes.bfloat16

B, T, D, H = 4, 2048, 1024, 16
HD = 64          # head dim
HL = 8           # heads per core
DL = 512         # attention dims per core
NKT = T // 128   # 16 k-tiles
NQC = T // 512   # 4 q-chunks
SCALE = HD ** -0.5
MASKVAL = -30000.0

F32 = mybir.dt.float32
F32R = mybir.dt.float32r
BF16 = mybir.dt.bfloat16
EXP = mybir.ActivationFunctionType.Exp

_cache = {}


def _build_nc():
    nc = bacc.Bacc("TRN2", target_bir_lowering=False, debug=False, num_devices=8)

    xT_d = nc.dram_tensor("xT", [D, T], BF16, kind="ExternalInput").ap()
    wqk_d = nc.dram_tensor("wqk", [D, 2 * DL], BF16, kind="ExternalInput").ap()
    wv_d = nc.dram_tensor("wv", [D, DL], BF16, kind="ExternalInput").ap()
    wo_d = nc.dram_tensor("wo", [DL, D], BF16, kind="ExternalInput").ap()
    cmask_d = nc.dram_tensor("cmask", [128, 4 * 512], F32, kind="ExternalInput").ap()
    sel_d = nc.dram_tensor("sel", [128, 4 * 64], F32, kind="ExternalInput").ap()
    outT_d = [
        nc.dram_tensor(f"outT{p}", [D, T], F32, kind="ExternalOutput").ap()
        for p in range(4)
    ]

    with tile.TileContext(nc) as tc:
        with (
            tc.tile_pool(name="persist", bufs=1) as pp,
            tc.tile_pool(name="work", bufs=1) as wp,
            tc.tile_pool(name="ps_m", bufs=2, space="PSUM") as ps_m,
            tc.tile_pool(name="ps_s", bufs=4, space="PSUM") as ps_s,
            tc.tile_pool(name="ps_o", bufs=2, space="PSUM") as ps_o,
        ):
            # ---- resident inputs -------------------------------------------------
            xT_sb = [pp.tile([128, T], BF16, tag=f"xT{i}", name=f"xT{i}") for i in range(8)]
            wqk_sb = [pp.tile([128, 2 * DL], BF16, tag=f"wqk{i}", name=f"wqk{i}") for i in range(8)]
            wv_sb = [pp.tile([128, DL], BF16, tag=f"wv{i}", name=f"wv{i}") for i in range(8)]
            wo_sb = [pp.tile([128, D], BF16, tag=f"wo{i}", name=f"wo{i}") for i in range(4)]
            cmask_sb = pp.tile([128, 4 * 512], F32, tag="cmask", name="cmask")
            sel_sb = pp.tile([128, 4 * 64], F32R, tag="sel", name="sel")

            for i in range(8):
                nc.sync.dma_start(wqk_sb[i][:], wqk_d[i * 128:(i + 1) * 128, :])
            for i in range(8):
                nc.sync.dma_start(xT_sb[i][:], xT_d[i * 128:(i + 1) * 128, :])
            for i in range(8):
                nc.sync.dma_start(wv_sb[i][:], wv_d[i * 128:(i + 1) * 128, :])
            for i in range(4):
                nc.sync.dma_start(wo_sb[i][:], wo_d[i * 128:(i + 1) * 128, :])
            nc.sync.dma_start(cmask_sb[:], cmask_d)
            nc.sync.dma_start(sel_sb[:], sel_d.bitcast(F32R))

            # ---- persistent intermediates ---------------------------------------
            qkT_sb = [pp.tile([128, T], BF16, tag=f"qkT{i}", name=f"qkT{i}") for i in range(8)]
            vaug_sb = [pp.tile([128, HL, HD + 1], BF16, tag=f"vaug{i}", name=f"vaug{i}") for i in range(NKT)]
            attnT_sb = [pp.tile([128, T], BF16, tag=f"attnT{i}", name=f"attnT{i}") for i in range(4)]

            for i in range(NKT):
                nc.vector.memset(vaug_sb[i][:, :, HD:HD + 1], 1.0)

            # ---- background unit emitters (each unit is PSUM-atomic) ------------
            def qk_chain(mt, qc):
                ps = ps_m.tile([128, 512], F32, tag="ps_m", name="ps_m")
                for kt in range(8):
                    nc.tensor.matmul(
                        ps[:],
                        wqk_sb[kt][:, mt * 128:(mt + 1) * 128],
                        xT_sb[kt][:, qc * 512:(qc + 1) * 512],
                        start=(kt == 0), stop=(kt == 7),
                    )
                nc.vector.tensor_copy(qkT_sb[mt][:, qc * 512:(qc + 1) * 512], ps[:])

            def v_chain(tt):
                ps = ps_m.tile([128, 512], F32, tag="ps_m", name="ps_m")
                for kt in range(8):
                    nc.tensor.matmul(
                        ps[:],
                        xT_sb[kt][:, tt * 128:(tt + 1) * 128],
                        wv_sb[kt][:],
                        start=(kt == 0), stop=(kt == 7),
                    )
                nc.vector.tensor_copy(
                    vaug_sb[tt][:, :, 0:HD],
                    ps[:].rearrange("p (h e) -> p h e", h=HL),
                )

            def outproj_unit(p, mt, qc):
                ps = ps_m.tile([128, 512], F32, tag="ps_m", name="ps_m")
                nc.tensor.matmul(
                    ps[:],
                    wo_sb[p][:, mt * 128:(mt + 1) * 128],
                    attnT_sb[p][:, qc * 512:(qc + 1) * 512],
                    start=True, stop=True,
                )
                st = wp.tile([128, 512], F32, tag="outst", bufs=3, name="outst")
                nc.any.tensor_copy(st[:], ps[:])
                nc.sync.dma_start(
                    outT_d[p][mt * 128:(mt + 1) * 128, qc * 512:(qc + 1) * 512],
                    st[:],
                )

            def bg_units(p):
                units = []
                if p == 0:
                    units += [(v_chain, (tt,)) for tt in range(8, NKT)]
                if p < 3:
                    for mt in (p + 1, 4 + p + 1):
                        for qc in range(NQC):
                            units.append((qk_chain, (mt, qc)))
                if p > 0:
                    for mt in range(8):
                        for qc in range(NQC):
                            units.append((outproj_unit, (p - 1, mt, qc)))
                return units

            # v[8..15] is consumed by PV step kt during pair 0; pace its chains
            # ahead of everything else so dependencies never stall the PE.

            # ---- warmup: qk for pair 0 and the first half of v ------------------
            for qc in range(NQC):
                qk_chain(0, qc)
                qk_chain(4, qc)
            for tt in range(8):
                v_chain(tt)

            # ---- attention: head pairs, 2-kt score bursts, bg interleave --------
            for p in range(4):
                qT = qkT_sb[p]
                kT = qkT_sb[4 + p]
                units = bg_units(p)
                nsteps = sum(4 * qc + 4 for qc in range(NQC))  # 40
                ustep = 0
                emitted = 0

                dens = {}
                numers = {}
                for ho in (0, 64):
                    dens[ho] = wp.tile([128, 512], F32, tag=f"den{ho}", bufs=2, name=f"den{ho}")
                    nc.vector.memset(dens[ho][:], 1.0)

                for qc in range(NQC):
                    po = {}
                    for ho in (0, 64):
                        po[ho] = ps_o.tile([128, 512], F32, tag="ps_o", name="ps_o")
                    last_kt = 4 * qc + 3
                    for kt0 in range(0, last_kt + 1, 2):
                        kts = [kt for kt in (kt0, kt0 + 1) if kt <= last_kt]
                        # score burst: K=64 matmuls alternating row groups
                        ets = {}
                        for kt in kts:
                            for ho in (0, 64):
                                ps = ps_s.tile([128, 512], F32, tag="ps_s", name="ps_s")
                                ets[(kt, ho)] = (
                                    ps,
                                    wp.tile([128, 512], BF16, tag="expT", bufs=8, name="expT"),
                                )
                                nc.tensor.matmul(
                                    ps[:],
                                    kT[ho:ho + 64, kt * 128:(kt + 1) * 128],
                                    qT[ho:ho + 64, qc * 512:(qc + 1) * 512],
                                    start=True, stop=True,
                                )
                        for kt in kts:
                            j = kt - 4 * qc
                            for ho in (0, 64):
                                ps, et = ets[(kt, ho)]
                                if 0 <= j <= 3:
                                    nc.vector.tensor_add(
                                        ps[:], ps[:], cmask_sb[:, j * 512:(j + 1) * 512]
                                    )
                                nc.scalar.activation(et[:], ps[:], EXP)
                        # background work hides the exp latency before PV
                        ustep += len(kts)
                        while units and emitted < len(units) and ustep * len(units) >= (emitted + 1) * nsteps:
                            fn, args = units[emitted]
                            fn(*args)
                            emitted += 1
                        for kt in kts:
                            for ho in (0, 64):
                                _, et = ets[(kt, ho)]
                                h = 2 * p + (1 if ho else 0)
                                nc.tensor.matmul(
                                    po[ho][0:HD + 1, :],
                                    vaug_sb[kt][:, h, :],
                                    et[:],
                                    start=(kt == 0), stop=(kt == last_kt),
                                )
                                if kt == last_kt:
                                    nm = wp.tile([64, 512], F32, tag=f"nm{ho}_{qc}",
                                                 bufs=2, name=f"nm{ho}_{qc}")
                                    numers[(ho, qc)] = nm
                                    nc.vector.tensor_copy(nm[:], po[ho][0:64, :])
                                    nc.vector.tensor_copy(
                                        dens[ho][32 * qc:32 * qc + 1, :],
                                        po[ho][HD:HD + 1, :],
                                    )
                # normalize both heads of the pair
                for ho in (0, 64):
                    rec = wp.tile([128, 512], F32R, tag="rec", bufs=2, name="rec")
                    with nc.allow_low_precision(reason="f32r reciprocal for denom broadcast"):
                        nc.vector.reciprocal(rec[:], dens[ho][:])
                    for qc in range(NQC):
                        pb = ps_m.tile([128, 512], F32, tag="ps_m", name="ps_m")
                        nc.tensor.matmul(
                            pb[0:64, :],
                            sel_sb[:, qc * 64:(qc + 1) * 64],
                            rec[:],
                            start=True, stop=True,
                        )
                        nc.vector.tensor_mul(
                            attnT_sb[p][ho:ho + 64, qc * 512:(qc + 1) * 512],
                            numers[(ho, qc)][:],
                            pb[0:64, :],
                        )
                while units and emitted < len(units):
                    fn, args = units[emitted]
                    fn(*args)
                    emitted += 1

            # ---- final partial output projection (pair 3) -----------------------
            for mt in range(8):
                for qc in range(NQC):
                    outproj_unit(3, mt, qc)

    nc.compile()
    return nc


def _host_shards(x, qkv_w, out_w):
    cmask = np.zeros((128, 4 * 512), np.float32)
    kp = np.arange(128)[:, None]
    qf = np.arange(512)[None, :]
    for j in range(4):
        cmask[:, j * 512:(j + 1) * 512] = np.where(kp <= qf - 128 * j, 0.0, MASKVAL)
    sel = np.zeros((128, 4 * 64), np.float32)
    for r in range(4):
        sel[32 * r, r * 64:(r + 1) * 64] = 1.0

    in_maps = []
    for c in range(8):
        b, hg = c // 2, c % 2
        xT = np.ascontiguousarray(x[b].T).astype(BF)
        qs = (qkv_w[hg * DL:(hg + 1) * DL] * np.float32(SCALE))
        ks = qkv_w[D + hg * DL:D + (hg + 1) * DL]
        wqk = np.ascontiguousarray(np.concatenate([qs, ks], 0).T).astype(BF)
        wv = np.ascontiguousarray(qkv_w[2 * D + hg * DL:2 * D + (hg + 1) * DL].T).astype(BF)
        wo = np.ascontiguousarray(out_w[:, hg * DL:(hg + 1) * DL].T).astype(BF)
        in_maps.append({
            "xT": xT,
            "wqk": wqk,
            "wv": wv,
            "wo": wo,
            "cmask": cmask,
            "sel": sel,
        })
    return in_maps


def kernel(x, qkv_w, out_w, out_b, _trace=False, _trace_kwargs=None):
    if "nc" not in _cache:
        _cache["nc"] = _build_nc()
    nc = _cache["nc"]
    in_maps = _host_shards(
        np.asarray(x, np.float32), np.asarray(qkv_w, np.float32),
        np.asarray(out_w, np.float32),
    )
    res = run_bass_kernel_spmd(
        nc, in_maps, core_ids=list(range(8)), trace=_trace,
        **(_trace_kwargs or {}),
    )
    _cache["last_result"] = res
    out = np.empty((B, T, D), np.float32)
    ob = np.asarray(out_b, np.float32)[None, :]
    for b in range(B):
        acc = None
        for c in (2 * b, 2 * b + 1):
            for p in range(4):
                t = res.results[c][f"outT{p}"]
                acc = t if acc is None else acc + t
        out[b] = acc.T + ob
    return out


# revision 22
# speedup vs baseline: 1.5569x; 1.0415x over previous
"""Causal multi-head attention block (qkv proj + attention + out proj) on 8 TRN2 cores.

Sharding: core c = (batch b = c//2, head-group hg = c%2 of 8 heads).
Each core computes, for its (b, hg): qk projection (transposed), v projection,
per-head causal softmax attention (transposed layout, denominator folded into
the PV matmul as an extra ones-row of v), and per-pair partial output
projections over 128 attention dims each. Host sums the 4 pair-partials of
both head-group cores per batch and adds the bias.

Attention processes heads in pairs (even head on partitions 0-63, odd head on
64-127). Score matmuls are K=64 and run in 2-kt bursts of 4 so consecutive
matmuls alternate PE row groups and overlap in the array. Projection chains
(qk for the next pair, out-projection partials for the previous pair, the
tail of the v projection) are interleaved into the attention steps to keep
the PE dense and HAM-warm while ACT runs the exps.
"""

import numpy as np
import ml_dtypes

import concourse.bass as bass
import concourse.bacc as bacc
import concourse.mybir as mybir
import concourse.tile as tile
from concourse.bass_utils import run_bass_kernel_spmd

BF = ml_dtypes.bfloat16

B, T, D, H = 4, 2048, 1024, 16
HD = 64          # head dim
HL = 8           # heads per core
DL = 512         # attention dims per core
NKT = T // 128   # 16 k-tiles
NQC = T // 512   # 4 q-chunks
SCALE = HD ** -0.5
MASKVAL = -30000.0

F32 = mybir.dt.float32
F32R = mybir.dt.float32r
BF16 = mybir.dt.bfloat16
EXP = mybir.ActivationFunctionType.Exp

_cache = {}


def _build_nc():
    nc = bacc.Bacc("TRN2", target_bir_lowering=False, debug=False, num_devices=8)

    xT_d = nc.dram_tensor("xT", [D, T], BF16, kind="ExternalInput").ap()
    wqk_d = nc.dram_tensor("wqk", [D, 2 * DL], BF16, kind="ExternalInput").ap()
    wv_d = nc.dram_tensor("wv", [D, DL], BF16, kind="ExternalInput").ap()
    wo_d = nc.dram_tensor("wo", [DL, D], BF16, kind="ExternalInput").ap()
    cmask_d = nc.dram_tensor("cmask", [128, 4 * 512], F32, kind="ExternalInput").ap()
    sel_d = nc.dram_tensor("sel", [128, 4 * 64], F32, kind="ExternalInput").ap()
    outT_d = [
        nc.dram_tensor(f"outT{p}", [D, T], F32, kind="ExternalOutput").ap()
        for p in range(4)
    ]

    with tile.TileContext(nc) as tc:
        with (
            tc.tile_pool(name="persist", bufs=1) as pp,
            tc.tile_pool(name="work", bufs=1) as wp,
            tc.tile_pool(name="ps_m", bufs=2, space="PSUM") as ps_m,
            tc.tile_pool(name="ps_s", bufs=4, space="PSUM") as ps_s,
            tc.tile_pool(name="ps_o", bufs=2, space="PSUM") as ps_o,
        ):
            # ---- resident inputs -------------------------------------------------
            xT_sb = [pp.tile([128, T], BF16, tag=f"xT{i}", name=f"xT{i}") for i in range(8)]
            wqk_sb = [pp.tile([128, 2 * DL], BF16, tag=f"wqk{i}", name=f"wqk{i}") for i in range(8)]
            wv_sb = [pp.tile([128, DL], BF16, tag=f"wv{i}", name=f"wv{i}") for i in range(8)]
            wo_sb = [pp.tile([128, D], BF16, tag=f"wo{i}", name=f"wo{i}") for i in range(4)]
            cmask_sb = pp.tile([128, 4 * 512], F32, tag="cmask", name="cmask")
            sel_sb = pp.tile([128, 4 * 64], F32R, tag="sel", name="sel")

            for i in range(8):
                nc.sync.dma_start(wqk_sb[i][:], wqk_d[i * 128:(i + 1) * 128, :])
            for i in range(8):
                nc.sync.dma_start(xT_sb[i][:], xT_d[i * 128:(i + 1) * 128, :])
            for i in range(8):
                nc.sync.dma_start(wv_sb[i][:], wv_d[i * 128:(i + 1) * 128, :])
            for i in range(4):
                nc.sync.dma_start(wo_sb[i][:], wo_d[i * 128:(i + 1) * 128, :])
            nc.sync.dma_start(cmask_sb[:], cmask_d)
            nc.sync.dma_start(sel_sb[:], sel_d.bitcast(F32R))

            # ---- persistent intermediates ---------------------------------------
            qkT_sb = [pp.tile([128, T], BF16, tag=f"qkT{i}", name=f"qkT{i}") for i in range(8)]
            vaug_sb = [pp.tile([128, HL, HD + 1], BF16, tag=f"vaug{i}", name=f"vaug{i}") for i in range(NKT)]
            attnT_sb = [pp.tile([128, T], BF16, tag=f"attnT{i}", name=f"attnT{i}") for i in range(4)]

            for i in range(NKT):
                nc.vector.memset(vaug_sb[i][:, :, HD:HD + 1], 1.0)

            # ---- background unit emitters (each unit is PSUM-atomic) ------------
            def qk_chain(mt, qc):
                ps = ps_m.tile([128, 512], F32, tag="ps_m", name="ps_m")
                for kt in range(8):
                    nc.tensor.matmul(
                        ps[:],
                        wqk_sb[kt][:, mt * 128:(mt + 1) * 128],
                        xT_sb[kt][:, qc * 512:(qc + 1) * 512],
                        start=(kt == 0), stop=(kt == 7),
                    )
                nc.vector.tensor_copy(qkT_sb[mt][:, qc * 512:(qc + 1) * 512], ps[:])

            def v_chain(tt):
                ps = ps_m.tile([128, 512], F32, tag="ps_m", name="ps_m")
                for kt in range(8):
                    nc.tensor.matmul(
                        ps[:],
                        xT_sb[kt][:, tt * 128:(tt + 1) * 128],
                        wv_sb[kt][:],
                        start=(kt == 0), stop=(kt == 7),
                    )
                nc.vector.tensor_copy(
                    vaug_sb[tt][:, :, 0:HD],
                    ps[:].rearrange("p (h e) -> p h e", h=HL),
                )

            def outproj_unit(p, mt, qc):
                ps = ps_m.tile([128, 512], F32, tag="ps_m", name="ps_m")
                nc.tensor.matmul(
                    ps[:],
                    wo_sb[p][:, mt * 128:(mt + 1) * 128],
                    attnT_sb[p][:, qc * 512:(qc + 1) * 512],
                    start=True, stop=True,
                )
                st = wp.tile([128, 512], F32, tag="outst", bufs=3, name="outst")
                nc.any.tensor_copy(st[:], ps[:])
                nc.sync.dma_start(
                    outT_d[p][mt * 128:(mt + 1) * 128, qc * 512:(qc + 1) * 512],
                    st[:],
                )

            def bg_units(p):
                units = []
                if p == 0:
                    units += [(v_chain, (tt,)) for tt in range(8, NKT)]
                if p < 3:
                    for mt in (p + 1, 4 + p + 1):
                        for qc in range(NQC):
                            units.append((qk_chain, (mt, qc)))
                if p > 0:
                    for mt in range(8):
                        for qc in range(NQC):
                            units.append((outproj_unit, (p - 1, mt, qc)))
                return units

            # v[8..15] is consumed by PV step kt during pair 0; pace its chains
            # ahead of everything else so dependencies never stall the PE.

            # ---- warmup: qk for pair 0 and the first half of v ------------------
            for qc in range(NQC):
                qk_chain(0, qc)
                qk_chain(4, qc)
            for tt in range(8):
                v_chain(tt)

            # ---- attention: head pairs, 2-kt score bursts, bg interleave --------
            # normalization of a finished pair, emitted as background units of
            # the next pair so it does not stall the PE at pair boundaries
            def norm_unit(pp, dens_pp, numers_pp, ho):
                rec = wp.tile([128, 512], F32R, tag="rec", bufs=2, name="rec")
                with nc.allow_low_precision(reason="f32r reciprocal for denom broadcast"):
                    nc.vector.reciprocal(rec[:], dens_pp[ho][:])
                for qc in range(NQC):
                    pb = ps_m.tile([128, 512], F32, tag="ps_m", name="ps_m")
                    nc.tensor.matmul(
                        pb[0:64, :],
                        sel_sb[:, qc * 64:(qc + 1) * 64],
                        rec[:],
                        start=True, stop=True,
                    )
                    nc.vector.tensor_mul(
                        attnT_sb[pp][ho:ho + 64, qc * 512:(qc + 1) * 512],
                        numers_pp[(ho, qc)][:],
                        pb[0:64, :],
                    )

            prev_norm = []
            for p in range(4):
                qT = qkT_sb[p]
                kT = qkT_sb[4 + p]
                units = prev_norm + bg_units(p)
                prev_norm = []
                nsteps = sum(4 * qc + 4 for qc in range(NQC))  # 40
                ustep = 0
                emitted = 0

                dens = {}
                numers = {}
                for ho in (0, 64):
                    dens[ho] = wp.tile([128, 512], F32, tag=f"den{ho}", bufs=2, name=f"den{ho}")
                    nc.vector.memset(dens[ho][:], 1.0)

                for qc in range(NQC):
                    po = {}
                    for ho in (0, 64):
                        po[ho] = ps_o.tile([128, 512], F32, tag="ps_o", name="ps_o")
                    last_kt = 4 * qc + 3
                    for kt0 in range(0, last_kt + 1, 2):
                        kts = [kt for kt in (kt0, kt0 + 1) if kt <= last_kt]
                        # score burst: K=64 matmuls alternating row groups
                        ets = {}
                        for kt in kts:
                            for ho in (0, 64):
                                ps = ps_s.tile([128, 512], F32, tag="ps_s", name="ps_s")
                                ets[(kt, ho)] = (
                                    ps,
                                    wp.tile([128, 512], BF16, tag="expT", bufs=8, name="expT"),
                                )
                                nc.tensor.matmul(
                                    ps[:],
                                    kT[ho:ho + 64, kt * 128:(kt + 1) * 128],
                                    qT[ho:ho + 64, qc * 512:(qc + 1) * 512],
                                    start=True, stop=True,
                                )
                        for kt in kts:
                            j = kt - 4 * qc
                            for ho in (0, 64):
                                ps, et = ets[(kt, ho)]
                                if 0 <= j <= 3:
                                    # columns < 128j are fully in the future:
                                    # zero them in et and exp only the rest,
                                    # with the triangular mask added first
                                    w0 = 128 * j
                                    if w0:
                                        nc.vector.memset(et[:, 0:w0], 0.0)
                                    nc.vector.tensor_add(
                                        ps[:, w0:512], ps[:, w0:512],
                                        cmask_sb[:, j * 512 + w0:(j + 1) * 512],
                                    )
                                    nc.scalar.activation(et[:, w0:512], ps[:, w0:512], EXP)
                                else:
                                    nc.scalar.activation(et[:], ps[:], EXP)
                        # background work hides the exp latency before PV
                        ustep += len(kts)
                        while units and emitted < len(units) and ustep * len(units) >= (emitted + 1) * nsteps:
                            fn, args = units[emitted]
                            fn(*args)
                            emitted += 1
                        for kt in kts:
                            for ho in (0, 64):
                                _, et = ets[(kt, ho)]
                                h = 2 * p + (1 if ho else 0)
                                nc.tensor.matmul(
                                    po[ho][0:HD + 1, :],
                                    vaug_sb[kt][:, h, :],
                                    et[:],
                                    start=(kt == 0), stop=(kt == last_kt),
                                )
                                if kt == last_kt:
                                    nm = wp.tile([64, 512], F32, tag=f"nm{ho}_{qc}",
                                                 bufs=2, name=f"nm{ho}_{qc}")
                                    numers[(ho, qc)] = nm
                                    nc.vector.tensor_copy(nm[:], po[ho][0:64, :])
                                    nc.vector.tensor_copy(
                                        dens[ho][32 * qc:32 * qc + 1, :],
                                        po[ho][HD:HD + 1, :],
                                    )
                while units and emitted < len(units):
                    fn, args = units[emitted]
                    fn(*args)
                    emitted += 1
                # pair's normalization becomes background work of the next pair
                prev_norm = [
                    (norm_unit, (p, dens, numers, 0)),
                    (norm_unit, (p, dens, numers, 64)),
                ]

            # ---- pair-3 normalization + final partial output projection ---------
            for fn, args in prev_norm:
                fn(*args)
            for mt in range(8):
                for qc in range(NQC):
                    outproj_unit(3, mt, qc)

    nc.compile()
    return nc


def _host_shards(x, qkv_w, out_w):
    cmask = np.zeros((128, 4 * 512), np.float32)
    kp = np.arange(128)[:, None]
    qf = np.arange(512)[None, :]
    for j in range(4):
        cmask[:, j * 512:(j + 1) * 512] = np.where(kp <= qf - 128 * j, 0.0, MASKVAL)
    sel = np.zeros((128, 4 * 64), np.float32)
    for r in range(4):
        sel[32 * r, r * 64:(r + 1) * 64] = 1.0

    in_maps = []
    for c in range(8):
        b, hg = c // 2, c % 2
        xT = np.ascontiguousarray(x[b].T).astype(BF)
        qs = (qkv_w[hg * DL:(hg + 1) * DL] * np.float32(SCALE))
        ks = qkv_w[D + hg * DL:D + (hg + 1) * DL]
        wqk = np.ascontiguousarray(np.concatenate([qs, ks], 0).T).astype(BF)
        wv = np.ascontiguousarray(qkv_w[2 * D + hg * DL:2 * D + (hg + 1) * DL].T).astype(BF)
        wo = np.ascontiguousarray(out_w[:, hg * DL:(hg + 1) * DL].T).astype(BF)
        in_maps.append({
            "xT": xT,
            "wqk": wqk,
            "wv": wv,
            "wo": wo,
            "cmask": cmask,
            "sel": sel,
        })
    return in_maps


def kernel(x, qkv_w, out_w, out_b, _trace=False, _trace_kwargs=None):
    if "nc" not in _cache:
        _cache["nc"] = _build_nc()
    nc = _cache["nc"]
    in_maps = _host_shards(
        np.asarray(x, np.float32), np.asarray(qkv_w, np.float32),
        np.asarray(out_w, np.float32),
    )
    res = run_bass_kernel_spmd(
        nc, in_maps, core_ids=list(range(8)), trace=_trace,
        **(_trace_kwargs or {}),
    )
    _cache["last_result"] = res
    out = np.empty((B, T, D), np.float32)
    ob = np.asarray(out_b, np.float32)[None, :]
    for b in range(B):
        acc = None
        for c in (2 * b, 2 * b + 1):
            for p in range(4):
                t = res.results[c][f"outT{p}"]
                acc = t if acc is None else acc + t
        out[b] = acc.T + ob
    return out


# revision 25
# speedup vs baseline: 1.6203x; 1.0407x over previous
"""Causal multi-head attention block (qkv proj + attention + out proj) on 8 TRN2 cores.

Sharding: core c = (batch b = c//2, head-group hg = c%2 of 8 heads).
Each core computes, for its (b, hg): qk projection (transposed), v projection,
per-head causal softmax attention (transposed layout, denominator folded into
the PV matmul as an extra ones-row of v), and per-pair partial output
projections over 128 attention dims each. Host sums the 4 pair-partials of
both head-group cores per batch and adds the bias.

Attention processes heads in pairs (even head on partitions 0-63, odd head on
64-127). Score matmuls are K=64 and run in 2-kt bursts of 4 so consecutive
matmuls alternate PE row groups and overlap in the array. Projection chains
(qk for the next pair, out-projection partials for the previous pair, the
tail of the v projection) are interleaved into the attention steps to keep
the PE dense and HAM-warm while ACT runs the exps.
"""

import numpy as np
import ml_dtypes

import concourse.bass as bass
import concourse.bacc as bacc
import concourse.mybir as mybir
import concourse.tile as tile
from concourse.bass_utils import run_bass_kernel_spmd

BF = ml_dtypes.bfloat16

B, T, D, H = 4, 2048, 1024, 16
HD = 64          # head dim
HL = 8           # heads per core
DL = 512         # attention dims per core
NKT = T // 128   # 16 k-tiles
NQC = T // 512   # 4 q-chunks
SCALE = HD ** -0.5
MASKVAL = -30000.0

F32 = mybir.dt.float32
F32R = mybir.dt.float32r
BF16 = mybir.dt.bfloat16
EXP = mybir.ActivationFunctionType.Exp

_cache = {}


def _build_nc():
    nc = bacc.Bacc("TRN2", target_bir_lowering=False, debug=False, num_devices=8)

    xT_d = nc.dram_tensor("xT", [D, T], BF16, kind="ExternalInput").ap()
    wqk_d = nc.dram_tensor("wqk", [D, 2 * DL], BF16, kind="ExternalInput").ap()
    wv_d = nc.dram_tensor("wv", [D, DL], BF16, kind="ExternalInput").ap()
    wo_d = nc.dram_tensor("wo", [DL, D], BF16, kind="ExternalInput").ap()
    cmask_d = nc.dram_tensor("cmask", [128, 4 * 512], F32, kind="ExternalInput").ap()
    sel_d = nc.dram_tensor("sel", [128, 4 * 64], F32, kind="ExternalInput").ap()
    outT_d = [
        nc.dram_tensor(f"outT{p}", [D, T], F32, kind="ExternalOutput").ap()
        for p in range(4)
    ]

    with tile.TileContext(nc) as tc:
        with (
            tc.tile_pool(name="persist", bufs=1) as pp,
            tc.tile_pool(name="work", bufs=1) as wp,
            tc.tile_pool(name="ps_m", bufs=2, space="PSUM") as ps_m,
            tc.tile_pool(name="ps_s", bufs=4, space="PSUM") as ps_s,
            tc.tile_pool(name="ps_o", bufs=2, space="PSUM") as ps_o,
        ):
            # ---- resident inputs -------------------------------------------------
            xT_sb = [pp.tile([128, T], BF16, tag=f"xT{i}", name=f"xT{i}") for i in range(8)]
            wqk_sb = [pp.tile([128, 2 * DL], BF16, tag=f"wqk{i}", name=f"wqk{i}") for i in range(8)]
            wv_sb = [pp.tile([128, DL], BF16, tag=f"wv{i}", name=f"wv{i}") for i in range(8)]
            wo_sb = [pp.tile([128, D], BF16, tag=f"wo{i}", name=f"wo{i}") for i in range(4)]
            cmask_sb = pp.tile([128, 4 * 512], F32, tag="cmask", name="cmask")
            sel_sb = pp.tile([128, 4 * 64], F32R, tag="sel", name="sel")

            for i in range(8):
                nc.sync.dma_start(wqk_sb[i][:], wqk_d[i * 128:(i + 1) * 128, :])
                nc.sync.dma_start(xT_sb[i][:], xT_d[i * 128:(i + 1) * 128, :])
            for i in range(8):
                nc.sync.dma_start(wv_sb[i][:], wv_d[i * 128:(i + 1) * 128, :])
            for i in range(4):
                nc.sync.dma_start(wo_sb[i][:], wo_d[i * 128:(i + 1) * 128, :])
            nc.sync.dma_start(cmask_sb[:], cmask_d)
            nc.sync.dma_start(sel_sb[:], sel_d.bitcast(F32R))

            # ---- persistent intermediates ---------------------------------------
            qkT_sb = [pp.tile([128, T], BF16, tag=f"qkT{i}", name=f"qkT{i}") for i in range(8)]
            vaug_sb = [pp.tile([128, HL, HD + 1], BF16, tag=f"vaug{i}", name=f"vaug{i}") for i in range(NKT)]
            attnT_sb = [pp.tile([128, T], BF16, tag=f"attnT{i}", name=f"attnT{i}") for i in range(4)]

            for i in range(NKT):
                nc.vector.memset(vaug_sb[i][:, :, HD:HD + 1], 1.0)

            # ---- background unit emitters (each unit is PSUM-atomic) ------------
            def qk_chain(mt, qc):
                ps = ps_m.tile([128, 512], F32, tag="ps_m", name="ps_m")
                for kt in range(8):
                    nc.tensor.matmul(
                        ps[:],
                        wqk_sb[kt][:, mt * 128:(mt + 1) * 128],
                        xT_sb[kt][:, qc * 512:(qc + 1) * 512],
                        start=(kt == 0), stop=(kt == 7),
                    )
                nc.vector.tensor_copy(qkT_sb[mt][:, qc * 512:(qc + 1) * 512], ps[:])

            def v_chain(tt):
                ps = ps_m.tile([128, 512], F32, tag="ps_m", name="ps_m")
                for kt in range(8):
                    nc.tensor.matmul(
                        ps[:],
                        xT_sb[kt][:, tt * 128:(tt + 1) * 128],
                        wv_sb[kt][:],
                        start=(kt == 0), stop=(kt == 7),
                    )
                nc.vector.tensor_copy(
                    vaug_sb[tt][:, :, 0:HD],
                    ps[:].rearrange("p (h e) -> p h e", h=HL),
                )

            def outproj_unit(p, mt, qc):
                ps = ps_m.tile([128, 512], F32, tag="ps_m", name="ps_m")
                nc.tensor.matmul(
                    ps[:],
                    wo_sb[p][:, mt * 128:(mt + 1) * 128],
                    attnT_sb[p][:, qc * 512:(qc + 1) * 512],
                    start=True, stop=True,
                )
                st = wp.tile([128, 512], F32, tag="outst", bufs=3, name="outst")
                nc.any.tensor_copy(st[:], ps[:])
                nc.sync.dma_start(
                    outT_d[p][mt * 128:(mt + 1) * 128, qc * 512:(qc + 1) * 512],
                    st[:],
                )

            def bg_units(p):
                units = []
                if p == 0:
                    units += [(v_chain, (tt,)) for tt in range(8, NKT)]
                if p < 3:
                    for mt in (p + 1, 4 + p + 1):
                        for qc in range(NQC):
                            units.append((qk_chain, (mt, qc)))
                if p > 0:
                    for mt in range(8):
                        for qc in range(NQC):
                            units.append((outproj_unit, (p - 1, mt, qc)))
                return units

            # v[8..15] is consumed by PV step kt during pair 0; pace its chains
            # ahead of everything else so dependencies never stall the PE.

            # ---- warmup: qk for pair 0 and the first half of v ------------------
            for qc in range(NQC):
                qk_chain(0, qc)
                qk_chain(4, qc)
            for tt in range(8):
                v_chain(tt)

            # ---- attention: head pairs, 2-kt score bursts, bg interleave --------
            # normalization of a finished pair, emitted as background units of
            # the next pair so it does not stall the PE at pair boundaries
            def norm_unit(pp, dens_pp, numers_pp, ho):
                rec = wp.tile([128, 512], F32R, tag="rec", bufs=2, name="rec")
                with nc.allow_low_precision(reason="f32r reciprocal for denom broadcast"):
                    nc.vector.reciprocal(rec[:], dens_pp[ho][:])
                for qc in range(NQC):
                    pb = ps_m.tile([128, 512], F32, tag="ps_m", name="ps_m")
                    nc.tensor.matmul(
                        pb[0:64, :],
                        sel_sb[:, qc * 64:(qc + 1) * 64],
                        rec[:],
                        start=True, stop=True,
                    )
                    nc.vector.tensor_mul(
                        attnT_sb[pp][ho:ho + 64, qc * 512:(qc + 1) * 512],
                        numers_pp[(ho, qc)][:],
                        pb[0:64, :],
                    )

            prev_norm = []
            for p in range(4):
                qT = qkT_sb[p]
                kT = qkT_sb[4 + p]
                units = prev_norm + bg_units(p)
                prev_norm = []
                nsteps = sum(4 * qc + 4 for qc in range(NQC))  # 40
                ustep = 0
                emitted = 0

                dens = {}
                numers = {}
                for ho in (0, 64):
                    dens[ho] = wp.tile([128, 512], F32, tag=f"den{ho}", bufs=2, name=f"den{ho}")
                    nc.vector.memset(dens[ho][:], 1.0)

                for qc in range(NQC):
                    po = {}
                    for ho in (0, 64):
                        po[ho] = ps_o.tile([128, 512], F32, tag="ps_o", name="ps_o")
                    last_kt = 4 * qc + 3
                    for kt0 in range(0, last_kt + 1, 2):
                        kts = [kt for kt in (kt0, kt0 + 1) if kt <= last_kt]
                        # score burst: K=64 matmuls alternating row groups.
                        # On diagonal tiles (j = kt - 4qc in 0..3) only the
                        # last 512-128j columns can be unmasked; scores, mask,
                        # exp, and PV all restrict to that window.
                        ets = {}
                        for kt in kts:
                            j = kt - 4 * qc
                            w0 = 128 * j if 0 <= j <= 3 else 0
                            for ho in (0, 64):
                                ps = ps_s.tile([128, 512], F32, tag="ps_s", name="ps_s")
                                ets[(kt, ho)] = (
                                    ps,
                                    wp.tile([128, 512], BF16, tag="expT", bufs=8, name="expT"),
                                    w0,
                                )
                                nc.tensor.matmul(
                                    ps[:, w0:512],
                                    kT[ho:ho + 64, kt * 128:(kt + 1) * 128],
                                    qT[ho:ho + 64, qc * 512 + w0:(qc + 1) * 512],
                                    start=True, stop=True,
                                )
                        for kt in kts:
                            j = kt - 4 * qc
                            for ho in (0, 64):
                                ps, et, w0 = ets[(kt, ho)]
                                if 0 <= j <= 3:
                                    nc.vector.tensor_add(
                                        ps[:, w0:512], ps[:, w0:512],
                                        cmask_sb[:, j * 512 + w0:(j + 1) * 512],
                                    )
                                nc.scalar.activation(et[:, w0:512], ps[:, w0:512], EXP)
                        # background work hides the exp latency before PV
                        ustep += len(kts)
                        while units and emitted < len(units) and ustep * len(units) >= (emitted + 1) * nsteps:
                            fn, args = units[emitted]
                            fn(*args)
                            emitted += 1
                        for kt in kts:
                            for ho in (0, 64):
                                _, et, w0 = ets[(kt, ho)]
                                h = 2 * p + (1 if ho else 0)
                                nc.tensor.matmul(
                                    po[ho][0:HD + 1, w0:512],
                                    vaug_sb[kt][:, h, :],
                                    et[:, w0:512],
                                    start=(kt == 0), stop=(kt == last_kt),
                                )
                                if kt == last_kt:
                                    nm = wp.tile([64, 512], F32, tag=f"nm{ho}_{qc}",
                                                 bufs=2, name=f"nm{ho}_{qc}")
                                    numers[(ho, qc)] = nm
                                    nc.vector.tensor_copy(nm[:], po[ho][0:64, :])
                                    nc.vector.tensor_copy(
                                        dens[ho][32 * qc:32 * qc + 1, :],
                                        po[ho][HD:HD + 1, :],
                                    )
                while units and emitted < len(units):
                    fn, args = units[emitted]
                    fn(*args)
                    emitted += 1
                # pair's normalization becomes background work of the next pair
                prev_norm = [
                    (norm_unit, (p, dens, numers, 0)),
                    (norm_unit, (p, dens, numers, 64)),
                ]

            # ---- pair-3 normalization + final partial output projection ---------
            for fn, args in prev_norm:
                fn(*args)
            for mt in range(8):
                for qc in range(NQC):
                    outproj_unit(3, mt, qc)

    nc.compile()
    return nc


def _host_shards(x, qkv_w, out_w):
    cmask = np.zeros((128, 4 * 512), np.float32)
    kp = np.arange(128)[:, None]
    qf = np.arange(512)[None, :]
    for j in range(4):
        cmask[:, j * 512:(j + 1) * 512] = np.where(kp <= qf - 128 * j, 0.0, MASKVAL)
    sel = np.zeros((128, 4 * 64), np.float32)
    for r in range(4):
        sel[32 * r, r * 64:(r + 1) * 64] = 1.0

    in_maps = []
    for c in range(8):
        b, hg = c // 2, c % 2
        xT = np.ascontiguousarray(x[b].T).astype(BF)
        qs = (qkv_w[hg * DL:(hg + 1) * DL] * np.float32(SCALE))
        ks = qkv_w[D + hg * DL:D + (hg + 1) * DL]
        wqk = np.ascontiguousarray(np.concatenate([qs, ks], 0).T).astype(BF)
        wv = np.ascontiguousarray(qkv_w[2 * D + hg * DL:2 * D + (hg + 1) * DL].T).astype(BF)
        wo = np.ascontiguousarray(out_w[:, hg * DL:(hg + 1) * DL].T).astype(BF)
        in_maps.append({
            "xT": xT,
            "wqk": wqk,
            "wv": wv,
            "wo": wo,
            "cmask": cmask,
            "sel": sel,
        })
    return in_maps


def kernel(x, qkv_w, out_w, out_b, _trace=False, _trace_kwargs=None):
    if "nc" not in _cache:
        _cache["nc"] = _build_nc()
    nc = _cache["nc"]
    in_maps = _host_shards(
        np.asarray(x, np.float32), np.asarray(qkv_w, np.float32),
        np.asarray(out_w, np.float32),
    )
    res = run_bass_kernel_spmd(
        nc, in_maps, core_ids=list(range(8)), trace=_trace,
        **(_trace_kwargs or {}),
    )
    _cache["last_result"] = res
    out = np.empty((B, T, D), np.float32)
    ob = np.asarray(out_b, np.float32)[None, :]
    for b in range(B):
        acc = None
        for c in (2 * b, 2 * b + 1):
            for p in range(4):
                t = res.results[c][f"outT{p}"]
                acc = t if acc is None else acc + t
        out[b] = acc.T + ob
    return out
